# revision 24
# baseline (speedup 1.0000x reference)
"""Single-head attention (B=4, S=4096, D=A=1024, fp32 I/O) on 8 TRN2 NeuronCores.

Sharding: core c handles batch b=c//2, sequence-half h=c%2 (2048 rows).
Each core projects Q, K^T and V for its own half only; core pairs exchange
K^T/V halves with chunked AllGathers (overlapped with projection compute), so
nothing is computed twice.  Attention then runs flash-style per 512-query
block against the full gathered sequence.

Q^T,K^T live as [A,S]-semantics (a on partitions), V as [S,A] (k on
partitions); scores are computed transposed ([k,q]); softmax normalization is
deferred to the output projection epilogue (exp without max subtraction is
safe here: scores are O(5)).  Matmul compute in bf16, accumulation fp32.
Softmax denominators accumulate on the vector engine (not the PE); one
cross-partition matmul per block finishes them.  k-tiles are enumerated in
gather order everywhere, which keeps scores, exp, sums and ctx consistent
without any index remapping.

v2 schedule notes (trace-driven):
- Initial loads split so the first K-proj matmul's deps (wk + x chunk 0)
  arrive on three parallel rings (~7.5us instead of ~16us).
- One 4-buffer epilogue staging rotation shared by K/V/Q projections: no
  staging WAR chain ever reaches the PE.
- wq rides the scalar ring so the V-exchange input stores on gpsimd are
  never head-of-line blocked (this previously delayed the V1 AllGather 21us).
- v_sb (gathered V) loads are emitted in PHASE 2 on the vector-engine queue:
  the phase-1 pool-exit drains no longer wait on the V collectives, so the
  first score block starts the moment Q-projection retires instead of 21us
  later.  Nothing latency-critical sits behind them on the vector queue.
- Score K^T tiles stream on sync+gpsimd alternately (one ring saturates at
  ~140GB/s, just under the 145GB/s demand), triple-buffered.
- Output stores alternate sync/scalar so the final block's 2MB doesn't
  serialize on one ring after the last matmul.
"""

import numpy as np
import ml_dtypes

import concourse.bass as bass
import concourse.tile as tile
from concourse import mybir
from concourse.bass_utils import run_bass_kernel_spmd

BF = mybir.dt.bfloat16
F32 = mybir.dt.float32
AF = mybir.ActivationFunctionType

B, S, DIM, A = 4, 4096, 1024, 1024
SQ = S // 2          # rows handled per core (query rows and local K/V rows)
NC = DIM // 128      # d chunks
NA = A // 128        # a tiles
NK = S // 128        # k tiles (global)
QB = 512             # q block width
NQB = SQ // QB
SCALE = 1.0 / np.sqrt(np.float32(A))

N_CORES = 8
PAIRS = [[0, 1], [2, 3], [4, 5], [6, 7]]

LAST_RESULT = None   # BassKernelResults of the most recent run (for test.py)


def _split_multiwaits(nc):
    """This walrus build rejects instructions carrying more than one sem wait
    (and Drains carrying any); hoist extra waits into single-wait NoOps
    preceding the instruction on the same engine."""
    for f in nc.m.functions:
        for bb in f.blocks:
            new_insts = []
            for inst in bb.instructions:
                si = inst.sync_info
                if si is not None and si.on_wait:
                    keep = 0 if isinstance(inst, mybir.InstDrain) else 1
                    if len(si.on_wait) > keep:
                        waits = list(si.on_wait)
                        hoist, rest = waits[: len(waits) - keep], waits[len(waits) - keep :]
                        for w in hoist:
                            nop = mybir.InstNoOp(
                                name=nc.get_next_instruction_name(),
                                sync_info=mybir.SyncInfo(on_wait=[w], on_update=[]),
                                bass_nofuse=True,
                                engine=inst.engine,
                            )
                            nc.register_instruction(nop)
                            new_insts.append(nop)
                        si.on_wait.clear()
                        si.on_wait.extend(rest)
                new_insts.append(inst)
            bb.instructions[:] = new_insts


def _build():
    nc = bass.Bass()

    # all pre-permuted host-side into [partition, ...contiguous...] layout
    xp = nc.declare_dram_parameter("xp", [128, 4, NC, 512], BF, isOutput=False)
    WkT = nc.declare_dram_parameter("WkT", [128, NC, A], BF, isOutput=False)
    WqT = nc.declare_dram_parameter("WqT", [128, NC, A], BF, isOutput=False)
    WvT = nc.declare_dram_parameter("WvT", [128, NC, A], BF, isOutput=False)
    WoT = nc.declare_dram_parameter("WoT", [128, NA, DIM], BF, isOutput=False)
    bqc = nc.declare_dram_parameter("bqc", [128, NA], F32, isOutput=False)
    bkc = nc.declare_dram_parameter("bkc", [128, NA], F32, isOutput=False)
    bvb = nc.declare_dram_parameter("bvb", [128, A], BF, isOutput=False)
    bob = nc.declare_dram_parameter("bob", [128, DIM], BF, isOutput=False)
    out = nc.declare_dram_parameter("out", [SQ, DIM], F32, isOutput=True)

    with tile.TileContext(nc) as tc:
        with (
            tc.tile_pool(name="dram", bufs=1, space="DRAM") as dram,
            tc.tile_pool(name="singles", bufs=1) as singles,
        ):
            # Q^T staging: [p, qb, c, q'] so stores and reloads are contiguous
            QT_d = dram.tile([128, NQB, NC, QB], BF, name="QT_d")
            # K^T exchange: [p, half, am, k'] per chunk; V: [p, j, a]
            kt_in = [
                dram.tile([128, 2, NA, 512], BF, name=f"kt_in{c}", tag=f"kti{c}")
                for c in range(2)
            ]
            kt_out = [
                dram.tile([2, 128, 2, NA, 512], BF, name=f"kt_out{c}", tag=f"kto{c}")
                for c in range(2)
            ]
            v_in = [
                dram.tile([128, 8, A], BF, name=f"v_in{c}", tag=f"vi{c}")
                for c in range(2)
            ]
            v_out = [
                dram.tile([2, 128, 8, A], BF, name=f"v_out{c}", tag=f"vo{c}")
                for c in range(2)
            ]

            warm_in = dram.tile([1, 128], BF, name="warm_in")
            warm_out = dram.tile([2, 1, 128], BF, name="warm_out")

            v_sb = singles.tile([128, NK, A], BF)        # V resident, 8.4 MB
            bqc_sb = singles.tile([128, NA], F32)
            bob_sb = singles.tile([128, DIM], BF)
            ones_k = singles.tile([128, 1], F32)         # sums matmul lhsT
            ones_1 = singles.tile([1, 1], BF)            # row->partition matmul rhs

            # phase-2 streaming pools allocated BEFORE the phase-1 pools so
            # their SBUF addresses are disjoint from phase-1 tiles -> their
            # prefetch DMAs carry no WAR dependency on phase-1 compute
            ksp = tc.tile_pool(name="p2k", bufs=3)
            p2k = ksp.__enter__()
            qtp = tc.tile_pool(name="p2q", bufs=3)
            p2q = qtp.__enter__()

            qt_pre = {}   # phase-2 Q tiles prefetched during phase 1
            ks_pre = {}   # phase-2 K^T tiles prefetched during phase 1

            # ---------------- Phase 1: projections + K/V exchange ----------
            with (
                tc.tile_pool(name="p1w", bufs=1) as p1w,
                tc.tile_pool(name="p1x", bufs=1) as p1x,
                tc.tile_pool(name="p1st", bufs=1) as p1st,
                tc.tile_pool(name="p1pk", bufs=2, space="PSUM") as p1pk,
                tc.tile_pool(name="p1pv", bufs=2, space="PSUM") as p1pv,
            ):
                wk = p1w.tile([128, NC, A], BF, tag="wkq")
                wv = p1w.tile([128, NC, A], BF, tag="wv")
                bkc_sb = p1st.tile([128, NA], F32, tag="bkc")
                bvb_sb = p1st.tile([128, A], BF, tag="bvb")
                # all of x^T stays resident through phase 1
                xs_all = p1x.tile([128, 4, NC, 512], BF)

                # one 3-deep epilogue staging rotation shared by K/V/Q: every
                # tag reuse is >=12us after the previous store retired, so no
                # WAR ever stalls an epilogue, and the last Q block bypasses
                # staging entirely (below)
                stage_n = [0]

                def stage_next(shape):
                    i = stage_n[0] % 3
                    stage_n[0] += 1
                    return p1st.tile(
                        shape, BF, tag=f"st{i}", name=f"st_u{stage_n[0]}_{i}"
                    )

                # wake the collectives firmware immediately (absorbs the
                # ~21us cross-core launch-skew barrier + ~25us cc startup
                # while the K projection is still computing)
                nc.sync.dma_start(out=warm_in[:], in_=xp[0:1, 0, 0, 0:128])
                nc.gpsimd.collective_compute(
                    "AllGather",
                    mybir.AluOpType.bypass,
                    replica_groups=PAIRS,
                    ins=[warm_in[:].opt()],
                    outs=[warm_out[:].opt()],
                )
                # initial loads: the first K-proj block needs wk (2MB) and
                # x chunk 0 (1MB); split 1MB-per-ring so it all lands ~19us
                # in (per-ring DMA sustains only ~105GB/s).  The scalar ring
                # carries NOTHING else before the kt_in exchange stores: any
                # queued junk there delays the first AllGather and with it
                # the whole firmware-serialized exchange chain.
                nc.scalar.dma_start(out=wk[:, 0:4, :], in_=WkT[:, 0:4, :])
                nc.scalar.dma_start(out=bkc_sb[:], in_=bkc[:])
                nc.sync.dma_start(out=wk[:, 4:8, :], in_=WkT[:, 4:8, :])
                nc.gpsimd.dma_start(out=xs_all[:, 0], in_=xp[:, 0])
                nc.sync.dma_start(out=xs_all[:, 1], in_=xp[:, 1])
                nc.gpsimd.dma_start(out=xs_all[:, 2], in_=xp[:, 2])
                nc.sync.dma_start(out=xs_all[:, 3], in_=xp[:, 3])
                nc.sync.dma_start(out=bvb_sb[:], in_=bvb[:])
                nc.sync.dma_start(out=bqc_sb[:], in_=bqc[:])
                nc.sync.dma_start(out=wv[:], in_=WvT[:])
                nc.sync.dma_start(out=bob_sb[:], in_=bob[:])
                nc.vector.memset(ones_k[:], 1.0)
                nc.vector.memset(ones_1[:], 1.0)

                def kt_chunk(c):
                    for sbl in range(2):
                        sb = c * 2 + sbl
                        kst = stage_next([128, NA, 512])
                        for am in range(NA):
                            pk = p1pk.tile([128, 512], F32)
                            for dc in range(NC):
                                nc.tensor.matmul(
                                    pk[:],
                                    lhsT=wk[:, dc, am * 128 : (am + 1) * 128],
                                    rhs=xs_all[:, sb, dc, :],
                                    start=(dc == 0),
                                    stop=(dc == NC - 1),
                                )
                            nc.scalar.activation(
                                kst[:, am, :], pk[:], AF.Identity,
                                bias=bkc_sb[:, am : am + 1],
                            )
                        nc.scalar.dma_start(out=kt_in[c][:, sbl], in_=kst[:])
                    nc.gpsimd.collective_compute(
                        "AllGather",
                        mybir.AluOpType.bypass,
                        replica_groups=PAIRS,
                        ins=[kt_in[c][:].opt()],
                        outs=[kt_out[c][:].opt()],
                    )

                def v_chunk(c):
                    for sbl in range(2):
                        sb = c * 2 + sbl
                        vst = stage_next([128, 4, 1024])
                        for st in range(4):
                            pv = p1pv.tile([128, 1024], F32)
                            for half in range(2):
                                for dc in range(NC):
                                    nc.tensor.matmul(
                                        pv[:, half * 512 : (half + 1) * 512],
                                        lhsT=xs_all[:, sb, dc, st * 128 : (st + 1) * 128],
                                        rhs=wv[:, dc, half * 512 : (half + 1) * 512],
                                        start=(dc == 0),
                                        stop=(dc == NC - 1),
                                    )
                            nc.vector.tensor_add(vst[:, st, :], pv[:], bvb_sb[:])
                        # V stores ride the gpsimd ring, which carries nothing
                        # collective-gated ahead of them -> the V AllGathers
                        # trigger the moment the data is computed
                        nc.gpsimd.dma_start(
                            out=v_in[c][:, sbl * 4 : (sbl + 1) * 4, :], in_=vst[:]
                        )
                    nc.gpsimd.collective_compute(
                        "AllGather",
                        mybir.AluOpType.bypass,
                        replica_groups=PAIRS,
                        ins=[v_in[c][:].opt()],
                        outs=[v_out[c][:].opt()],
                    )

                # K chunks first: the exchange chain is firmware-serialized,
                # and phase 2 needs K^T chunk 0 the moment scores start while
                # V isn't consumed until ~55/~135us into attention
                kt_chunk(0)
                kt_chunk(1)
                # wq reuses wk's buffer (tag "wkq"); its WAR dependency on the
                # last K-proj matmul rides the gpsimd ring and clears before
                # the V stores behind it have data anyway
                wq = p1w.tile([128, NC, A], BF, tag="wkq")
                nc.gpsimd.dma_start(out=wq[:], in_=WqT[:])
                v_chunk(0)
                v_chunk(1)

                # prefetch the first three K^T tiles of phase 2 (sync ring;
                # gated only on the first K exchange)
                for (hh, half) in ((0, 0), (0, 1), (1, 0)):
                    ks = p2k.tile([128, NC, 512], BF, name=f"ks0_0{hh}{half}", tag="ks")
                    nc.sync.dma_start(out=ks[:], in_=kt_out[0][hh, :, half])
                    ks_pre[(0, 0, hh, half)] = ks

                # --- Q projection (overlaps the V exchanges) ---
                # qb 0..2 stage + store + (for 0,1) reload-prefetch; the LAST
                # block writes its epilogue STRAIGHT into a pre-reserved p2q
                # tile: no staging, no QT store, no reload.  This pulls the
                # last sync-queue DMA to ~10us before the V1 AllGather opens,
                # so the phase-2 pool barrier (which waits on all DMA issued
                # before the phase-1 pool exit) clears the moment the PE
                # finishes, instead of crawling through the collective.
                for qb in range(NQB):
                    if qb == NQB - 1:
                        qst = p2q.tile([128, NC, QB], BF, name=f"qt{qb}", tag="qt")
                        qt_pre[qb] = qst
                    else:
                        qst = stage_next([128, NA, 512])
                    for am in range(NA):
                        pq = p1pk.tile([128, 512], F32)
                        for dc in range(NC):
                            nc.tensor.matmul(
                                pq[:],
                                lhsT=wq[:, dc, am * 128 : (am + 1) * 128],
                                rhs=xs_all[:, qb, dc, :],
                                start=(dc == 0),
                                stop=(dc == NC - 1),
                            )
                        nc.scalar.activation(
                            qst[:, am, :], pq[:], AF.Identity,
                            bias=bqc_sb[:, am : am + 1],
                        )
                    if qb < NQB - 1:
                        nc.sync.dma_start(out=QT_d[:, qb], in_=qst[:])
                        if qb < 2:
                            qt = p2q.tile([128, NC, QB], BF, name=f"qt{qb}", tag="qt")
                            nc.sync.dma_start(out=qt[:], in_=QT_d[:, qb])
                            qt_pre[qb] = qt

            # ---------------- Phase 2: attention ----------------
            with (
                tc.tile_pool(name="p2w", bufs=1) as p2w,
                tc.tile_pool(name="p2e", bufs=2) as p2e,
                tc.tile_pool(name="p2a", bufs=1) as p2a,
                tc.tile_pool(name="p2c", bufs=1) as p2c,
                tc.tile_pool(name="p2s", bufs=1) as p2s,
                tc.tile_pool(name="p2r", bufs=1) as p2r,
                tc.tile_pool(name="p2o", bufs=2) as p2o,
                tc.tile_pool(name="pps", bufs=2, space="PSUM") as pps,
                tc.tile_pool(name="ppsum", bufs=1, space="PSUM") as ppsum,
                tc.tile_pool(name="ppt", bufs=1, space="PSUM") as ppt,
                tc.tile_pool(name="ppc", bufs=2, space="PSUM") as ppc,
                tc.tile_pool(name="ppo", bufs=2, space="PSUM") as ppo,
            ):
                # Wo lives in the space freed by the phase-1 pools; it is not
                # needed until the first output projection (~110us later).
                # Split across sync+scalar so neither ring is busy >10us when
                # the first score K^T tiles start streaming.
                wo_sb = p2w.tile([128, NC, DIM], BF)
                nc.scalar.dma_start(out=wo_sb[:, 0:4, :], in_=WoT[:, 0:4, :])
                nc.sync.dma_start(out=wo_sb[:, 4:8, :], in_=WoT[:, 4:8, :])

                # gathered V -> resident SBUF.  Emitted AFTER the phase-1
                # pools exited (so the pool-exit drains don't wait on the V
                # collectives), and parked on the gpsimd queue, which carries
                # NOTHING else in phase 2: the Tile scheduler mis-models
                # collective completion times and will happily slot a
                # collective-gated trigger ahead of critical traffic on a
                # shared queue (that stalled phase 2 by 40us in v2).  On a
                # dedicated queue the dependency order (AG triggers first,
                # loads after) is forced and head-of-line blocking is free.
                for c in range(2):
                    for hh in range(2):
                        nc.gpsimd.dma_start(
                            out=v_sb[:, c * 16 + hh * 8 : c * 16 + hh * 8 + 8, :],
                            in_=v_out[c][hh, :, 0:8, :],
                        )

                ks_ring = [0]

                def do_scores(qb):
                    if qb in qt_pre:
                        qt = qt_pre.pop(qb)
                    else:
                        qt = p2q.tile([128, NC, QB], BF, name=f"qt{qb}", tag="qt")
                        nc.scalar.dma_start(out=qt[:], in_=QT_d[:, qb])
                    et = p2e.tile([128, NK, QB], BF, name=f"et{qb}", tag="et")
                    # per-partition partial softmax denominators accumulate on
                    # the vector engine as the exp tiles appear, so the PE
                    # only pays one cross-partition matmul per block
                    acc = p2a.tile([128, QB], F32, name=f"acc{qb}", tag="acc")
                    # scores^T + exp; k-tile groups of 4 share one KT load,
                    # alternating between the sync and gpsimd rings (one ring
                    # alone can't sustain the 145GB/s the PE consumes at)
                    for c in range(2):
                        for hh in range(2):
                            for half in range(2):
                                if (qb, c, hh, half) in ks_pre:
                                    ks = ks_pre.pop((qb, c, hh, half))
                                else:
                                    ks = p2k.tile([128, NC, 512], BF, name=f"ks{qb}_{c}{hh}{half}", tag="ks")
                                    eng = nc.sync if ks_ring[0] % 2 == 0 else nc.scalar
                                    ks_ring[0] += 1
                                    eng.dma_start(out=ks[:], in_=kt_out[c][hh, :, half])
                                ebase = c * 16 + hh * 8 + half * 4
                                for kt4 in range(4):
                                    ps = pps.tile([128, QB], F32, name=f"ps{qb}_{ebase+kt4}", tag="ps")
                                    for ac in range(NC):
                                        nc.tensor.matmul(
                                            ps[:],
                                            lhsT=ks[:, ac, kt4 * 128 : (kt4 + 1) * 128],
                                            rhs=qt[:, ac, :],
                                            start=(ac == 0),
                                            stop=(ac == NC - 1),
                                        )
                                    nc.scalar.activation(
                                        et[:, ebase + kt4, :],
                                        ps[:],
                                        AF.Exp,
                                        scale=float(SCALE),
                                    )
                                    kt = ebase + kt4
                                    if kt == 0:
                                        nc.vector.tensor_copy(acc[:], et[:, 0, :])
                                    else:
                                        nc.vector.tensor_add(
                                            acc[:], acc[:], et[:, kt, :]
                                        )
                    return et, acc

                def do_sums(qb, acc):
                    # softmax denominators: single cross-partition matmul on
                    # the DVE-accumulated partials, then row->partition
                    p_row = ppsum.tile([1, QB], F32, name=f"p_row{qb}", tag="p_row")
                    nc.tensor.matmul(
                        p_row[:],
                        lhsT=ones_k[:, 0:1],
                        rhs=acc[:],
                        start=True,
                        stop=True,
                    )
                    # bf16 denominators: 0.4% rounding on the softmax sums,
                    # well inside the error budget, and halves the tile
                    srow = p2s.tile([1, QB], BF, name=f"srow{qb}", tag="srow")
                    nc.scalar.copy(srow[:], p_row[:])
                    recips = p2r.tile([128, 4], F32, name=f"recips{qb}", tag="recips")
                    for qi in range(4):
                        ptt = ppt.tile([128, 1], F32, name=f"ptt{qb}_{qi}", tag="ptt")
                        nc.tensor.matmul(
                            ptt[:],
                            lhsT=srow[0:1, qi * 128 : (qi + 1) * 128],
                            rhs=ones_1[0:1, 0:1],
                            start=True,
                            stop=True,
                        )
                        nc.vector.reciprocal(recips[:, qi : qi + 1], ptt[:])
                    return recips

                def do_ctxA(qb, et):
                    # first gather half of ctx^T, written straight into the
                    # (single) ct buffer; ctxB accumulates in place
                    ct = p2c.tile([128, NA, QB], BF, name=f"ct{qb}", tag="ct")
                    for at in range(NA):
                        pc = ppc.tile([128, QB], F32, name=f"pcA{qb}_{at}", tag="pc")
                        for kt in range(NK // 2):
                            nc.tensor.matmul(
                                pc[:],
                                lhsT=v_sb[:, kt, at * 128 : (at + 1) * 128],
                                rhs=et[:, kt, :],
                                start=(kt == 0),
                                stop=(kt == NK // 2 - 1),
                            )
                        nc.vector.tensor_copy(ct[:, at, :], pc[:])
                    return ct

                def do_ctxB(qb, et, ct):
                    for at in range(NA):
                        pc = ppc.tile([128, QB], F32, name=f"pcB{qb}_{at}", tag="pc")
                        for kt in range(NK // 2, NK):
                            nc.tensor.matmul(
                                pc[:],
                                lhsT=v_sb[:, kt, at * 128 : (at + 1) * 128],
                                rhs=et[:, kt, :],
                                start=(kt == NK // 2),
                                stop=(kt == NK - 1),
                            )
                        nc.vector.tensor_add(ct[:, at, :], pc[:], ct[:, at, :])
                    return ct

                def do_out(qb, ct, recips):
                    # output projection + deferred softmax normalization + bias
                    for qi in range(4):
                        for half in range(2):
                            po = ppo.tile([128, 512], F32, name=f"po{qb}_{qi}{half}", tag="po")
                            for ac in range(NC):
                                nc.tensor.matmul(
                                    po[:],
                                    lhsT=ct[:, ac, qi * 128 : (qi + 1) * 128],
                                    rhs=wo_sb[:, ac, half * 512 : (half + 1) * 512],
                                    start=(ac == 0),
                                    stop=(ac == NC - 1),
                                )
                            for h2 in range(2):
                                ob = p2o.tile([128, 256], F32, name=f"ob{qb}_{qi}{half}{h2}", tag="ob")
                                nc.vector.tensor_scalar(
                                    ob[:],
                                    po[:, h2 * 256 : (h2 + 1) * 256],
                                    recips[:, qi : qi + 1],
                                    None,
                                    op0=mybir.AluOpType.mult,
                                )
                                off = half * 512 + h2 * 256
                                nc.vector.tensor_add(
                                    ob[:], ob[:], bob_sb[:, off : off + 256]
                                )
                                eng = nc.sync if qi % 2 == 0 else nc.scalar
                                eng.dma_start(
                                    out=out[
                                        (qb * 4 + qi) * 128 : (qb * 4 + qi + 1) * 128,
                                        off : off + 256,
                                    ],
                                    in_=ob[:],
                                )

                # software pipeline: the next block's scores are emitted
                # between ctxA and ctxB of the current block, so the PE has
                # independent work while the ctx/out chain of the current
                # block is still settling.  do_sums sits after ctxA so its
                # matmul never waits on the tail of the DVE accumulation chain
                et0, acc0 = do_scores(0)
                cA0 = do_ctxA(0, et0)
                r0 = do_sums(0, acc0)
                et_next, acc_next = do_scores(1)
                ct0 = do_ctxB(0, et0, cA0)
                do_out(0, ct0, r0)
                for qb in range(1, NQB):
                    et, acc = et_next, acc_next
                    cA = do_ctxA(qb, et)
                    r = do_sums(qb, acc)
                    if qb + 1 < NQB:
                        et_next, acc_next = do_scores(qb + 1)
                    ct = do_ctxB(qb, et, cA)
                    do_out(qb, ct, r)
            qtp.__exit__(None, None, None)
            ksp.__exit__(None, None, None)

    _split_multiwaits(nc)
    return nc


_NC_CACHE = None


def _get_nc():
    global _NC_CACHE
    if _NC_CACHE is None:
        _NC_CACHE = _build()
    return _NC_CACHE


def kernel(x, Wq, bq, Wk, bk, Wv, bv, Wo, bo):
    global LAST_RESULT
    bf16 = ml_dtypes.bfloat16
    x = np.asarray(x, np.float32)

    def permw(w):
        # [out(=rows of W^T after .T), in] -> W^T [in, out] -> [128, in/128, out]
        wT = np.asarray(w, np.float32).T
        return np.ascontiguousarray(
            wT.reshape(NC, 128, wT.shape[1]).transpose(1, 0, 2)
        ).astype(bf16)

    WqTp = permw(Wq)
    WkTp = permw(Wk)
    WvTp = permw(Wv)
    WoTp = permw(Wo)
    bqc = np.ascontiguousarray(np.asarray(bq, np.float32).reshape(NA, 128).T)
    bkc = np.ascontiguousarray(np.asarray(bk, np.float32).reshape(NA, 128).T)
    bvb = np.ascontiguousarray(np.broadcast_to(np.asarray(bv, np.float32), (128, A))).astype(bf16)
    bob = np.ascontiguousarray(np.broadcast_to(np.asarray(bo, np.float32), (128, DIM))).astype(bf16)

    in_maps = []
    for c in range(N_CORES):
        b, h = c // 2, c % 2
        xTq = x[b, h * SQ : (h + 1) * SQ, :].T  # [DIM, SQ]
        # [dc*128+p, sb*512+s] -> [p, sb, dc, s]
        xp = np.ascontiguousarray(
            xTq.reshape(NC, 128, 4, 512).transpose(1, 2, 0, 3)
        ).astype(bf16)
        in_maps.append(
            {
                "xp": xp,
                "WqT": WqTp,
                "WkT": WkTp,
                "WvT": WvTp,
                "WoT": WoTp,
                "bqc": bqc,
                "bkc": bkc,
                "bvb": bvb,
                "bob": bob,
            }
        )

    nc = _get_nc()
    import os

    res = run_bass_kernel_spmd(
        nc,
        in_maps,
        core_ids=list(range(N_CORES)),
        trace=bool(os.environ.get("BASS_TRACE")),
    )
    LAST_RESULT = res

    out_full = np.empty((B, S, DIM), np.float32)
    for c in range(N_CORES):
        b, h = c // 2, c % 2
        out_full[b, h * SQ : (h + 1) * SQ, :] = np.asarray(
            res.results[c]["out"], dtype=np.float32
        )
    return out_full


# revision 27
# speedup vs baseline: 1.1249x; 1.1249x over previous
"""Single-head attention (B=4, S=4096, D=A=1024, fp32 I/O) on 8 TRN2 NeuronCores.

Sharding: core c handles batch b=c//2, sequence-half h=c%2 (2048 rows).
Each core projects Q, K^T and V for its own half only; core pairs exchange
K^T/V halves with chunked AllGathers (overlapped with projection compute), so
nothing is computed twice.  Attention then runs flash-style per 512-query
block against the full gathered sequence.

Q^T,K^T live as [A,S]-semantics (a on partitions), V as [S,A] (k on
partitions); scores are computed transposed ([k,q]); softmax normalization is
deferred to the output projection epilogue (exp without max subtraction is
safe here: scores are O(5)).  Matmul compute in bf16, accumulation fp32.
Softmax denominators accumulate on the vector engine (not the PE); one
cross-partition matmul per block finishes them.  k-tiles are enumerated in
gather order everywhere, which keeps scores, exp, sums and ctx consistent
without any index remapping.

v2 schedule notes (trace-driven):
- Initial loads split so the first K-proj matmul's deps (wk + x chunk 0)
  arrive on three parallel rings (~7.5us instead of ~16us).
- One 4-buffer epilogue staging rotation shared by K/V/Q projections: no
  staging WAR chain ever reaches the PE.
- wq rides the scalar ring so the V-exchange input stores on gpsimd are
  never head-of-line blocked (this previously delayed the V1 AllGather 21us).
- v_sb (gathered V) loads are emitted in PHASE 2 on the vector-engine queue:
  the phase-1 pool-exit drains no longer wait on the V collectives, so the
  first score block starts the moment Q-projection retires instead of 21us
  later.  Nothing latency-critical sits behind them on the vector queue.
- Score K^T tiles stream on sync+gpsimd alternately (one ring saturates at
  ~140GB/s, just under the 145GB/s demand), triple-buffered.
- Output stores alternate sync/scalar so the final block's 2MB doesn't
  serialize on one ring after the last matmul.
"""

import numpy as np
import ml_dtypes

import concourse.bass as bass
import concourse.tile as tile
from concourse import mybir
from concourse.bass_utils import run_bass_kernel_spmd

BF = mybir.dt.bfloat16
F32 = mybir.dt.float32
AF = mybir.ActivationFunctionType

B, S, DIM, A = 4, 4096, 1024, 1024
SQ = S // 2          # rows handled per core (query rows and local K/V rows)
NC = DIM // 128      # d chunks
NA = A // 128        # a tiles
NK = S // 128        # k tiles (global)
QB = 512             # q block width
NQB = SQ // QB
SCALE = 1.0 / np.sqrt(np.float32(A))

N_CORES = 8
PAIRS = [[0, 1], [2, 3], [4, 5], [6, 7]]

LAST_RESULT = None   # BassKernelResults of the most recent run (for test.py)


def _split_multiwaits(nc):
    """This walrus build rejects instructions carrying more than one sem wait
    (and Drains carrying any); hoist extra waits into single-wait NoOps
    preceding the instruction on the same engine."""
    for f in nc.m.functions:
        for bb in f.blocks:
            new_insts = []
            for inst in bb.instructions:
                si = inst.sync_info
                if si is not None and si.on_wait:
                    keep = 0 if isinstance(inst, mybir.InstDrain) else 1
                    if len(si.on_wait) > keep:
                        waits = list(si.on_wait)
                        hoist, rest = waits[: len(waits) - keep], waits[len(waits) - keep :]
                        for w in hoist:
                            nop = mybir.InstNoOp(
                                name=nc.get_next_instruction_name(),
                                sync_info=mybir.SyncInfo(on_wait=[w], on_update=[]),
                                bass_nofuse=True,
                                engine=inst.engine,
                            )
                            nc.register_instruction(nop)
                            new_insts.append(nop)
                        si.on_wait.clear()
                        si.on_wait.extend(rest)
                new_insts.append(inst)
            bb.instructions[:] = new_insts


def _build():
    nc = bass.Bass()

    # all pre-permuted host-side into [partition, ...contiguous...] layout
    xp = nc.declare_dram_parameter("xp", [128, 4, NC, 512], BF, isOutput=False)
    WkT = nc.declare_dram_parameter("WkT", [128, NC, A], BF, isOutput=False)
    WqT = nc.declare_dram_parameter("WqT", [128, NC, A], BF, isOutput=False)
    WvT = nc.declare_dram_parameter("WvT", [128, NC, A], BF, isOutput=False)
    WoT = nc.declare_dram_parameter("WoT", [128, NA, DIM], BF, isOutput=False)
    bqc = nc.declare_dram_parameter("bqc", [128, NA], F32, isOutput=False)
    bkc = nc.declare_dram_parameter("bkc", [128, NA], F32, isOutput=False)
    bvb = nc.declare_dram_parameter("bvb", [128, A], BF, isOutput=False)
    bob = nc.declare_dram_parameter("bob", [128, DIM], BF, isOutput=False)
    out = nc.declare_dram_parameter("out", [SQ, DIM], F32, isOutput=True)

    with tile.TileContext(nc) as tc:
        with (
            tc.tile_pool(name="dram", bufs=1, space="DRAM") as dram,
            tc.tile_pool(name="singles", bufs=1) as singles,
        ):
            # Q^T staging: [p, qb, c, q'] so stores and reloads are contiguous
            QT_d = dram.tile([128, NQB, NC, QB], BF, name="QT_d")
            # K^T exchange: [p, half, am, k'] per chunk; V: [p, j, a]
            kt_in = [
                dram.tile([128, 2, NA, 512], BF, name=f"kt_in{c}", tag=f"kti{c}")
                for c in range(2)
            ]
            kt_out = [
                dram.tile([2, 128, 2, NA, 512], BF, name=f"kt_out{c}", tag=f"kto{c}")
                for c in range(2)
            ]
            v_in = [
                dram.tile([128, 8, A], BF, name=f"v_in{c}", tag=f"vi{c}")
                for c in range(2)
            ]
            v_out = [
                dram.tile([2, 128, 8, A], BF, name=f"v_out{c}", tag=f"vo{c}")
                for c in range(2)
            ]

            warm_in = dram.tile([1, 128], BF, name="warm_in")
            warm_out = dram.tile([2, 1, 128], BF, name="warm_out")

            v_sb = singles.tile([128, NK, A], BF)        # V resident, 8.4 MB
            bqc_sb = singles.tile([128, NA], F32)
            bob_sb = singles.tile([128, DIM], BF)
            ones_k = singles.tile([128, 1], F32)         # sums matmul lhsT
            ones_1 = singles.tile([1, 1], BF)            # row->partition matmul rhs

            # phase-2 streaming pools allocated BEFORE the phase-1 pools so
            # their SBUF addresses are disjoint from phase-1 tiles -> their
            # prefetch DMAs carry no WAR dependency on phase-1 compute
            ksp = tc.tile_pool(name="p2k", bufs=3)
            p2k = ksp.__enter__()
            qtp = tc.tile_pool(name="p2q", bufs=3)
            p2q = qtp.__enter__()

            qt_pre = {}   # phase-2 Q tiles prefetched during phase 1
            ks_pre = {}   # phase-2 K^T tiles prefetched during phase 1

            # ---------------- Phase 1: projections + K/V exchange ----------
            with (
                tc.tile_pool(name="p1w", bufs=1) as p1w,
                tc.tile_pool(name="p1x", bufs=1) as p1x,
                tc.tile_pool(name="p1st", bufs=1) as p1st,
                tc.tile_pool(name="p1pk", bufs=2, space="PSUM") as p1pk,
                tc.tile_pool(name="p1pv", bufs=2, space="PSUM") as p1pv,
            ):
                wk = p1w.tile([128, NC, A], BF, tag="wkq")
                wv = p1w.tile([128, NC, A], BF, tag="wv")
                bkc_sb = p1st.tile([128, NA], F32, tag="bkc")
                bvb_sb = p1st.tile([128, A], BF, tag="bvb")
                # all of x^T stays resident through phase 1
                xs_all = p1x.tile([128, 4, NC, 512], BF)

                # one 3-deep epilogue staging rotation shared by K/V/Q: every
                # tag reuse is >=12us after the previous store retired, so no
                # WAR ever stalls an epilogue, and the last Q block bypasses
                # staging entirely (below)
                stage_n = [0]

                def stage_next(shape):
                    i = stage_n[0] % 3
                    stage_n[0] += 1
                    return p1st.tile(
                        shape, BF, tag=f"st{i}", name=f"st_u{stage_n[0]}_{i}"
                    )

                # wake the collectives firmware immediately (absorbs the
                # ~21us cross-core launch-skew barrier + ~25us cc startup
                # while the K projection is still computing)
                nc.sync.dma_start(out=warm_in[:], in_=xp[0:1, 0, 0, 0:128])
                nc.gpsimd.collective_compute(
                    "AllGather",
                    mybir.AluOpType.bypass,
                    replica_groups=PAIRS,
                    ins=[warm_in[:].opt()],
                    outs=[warm_out[:].opt()],
                )
                # initial loads: the first K-proj block needs wk (2MB) and
                # x chunk 0 (1MB); split 1MB-per-ring so it all lands ~19us
                # in (per-ring DMA sustains only ~105GB/s).  The scalar ring
                # carries NOTHING else before the kt_in exchange stores: any
                # queued junk there delays the first AllGather and with it
                # the whole firmware-serialized exchange chain.
                nc.scalar.dma_start(out=wk[:, 0:4, :], in_=WkT[:, 0:4, :])
                nc.scalar.dma_start(out=bkc_sb[:], in_=bkc[:])
                nc.sync.dma_start(out=wk[:, 4:8, :], in_=WkT[:, 4:8, :])
                nc.gpsimd.dma_start(out=xs_all[:, 0], in_=xp[:, 0])
                nc.sync.dma_start(out=xs_all[:, 1], in_=xp[:, 1])
                nc.gpsimd.dma_start(out=xs_all[:, 2], in_=xp[:, 2])
                nc.sync.dma_start(out=xs_all[:, 3], in_=xp[:, 3])
                nc.sync.dma_start(out=bvb_sb[:], in_=bvb[:])
                nc.sync.dma_start(out=bqc_sb[:], in_=bqc[:])
                nc.sync.dma_start(out=wv[:], in_=WvT[:])
                nc.sync.dma_start(out=bob_sb[:], in_=bob[:])
                nc.vector.memset(ones_k[:], 1.0)
                nc.vector.memset(ones_1[:], 1.0)

                def kt_chunk(c):
                    for sbl in range(2):
                        sb = c * 2 + sbl
                        kst = stage_next([128, NA, 512])
                        for am in range(NA):
                            pk = p1pk.tile([128, 512], F32)
                            for dc in range(NC):
                                nc.tensor.matmul(
                                    pk[:],
                                    lhsT=wk[:, dc, am * 128 : (am + 1) * 128],
                                    rhs=xs_all[:, sb, dc, :],
                                    start=(dc == 0),
                                    stop=(dc == NC - 1),
                                )
                            nc.scalar.activation(
                                kst[:, am, :], pk[:], AF.Identity,
                                bias=bkc_sb[:, am : am + 1],
                            )
                        nc.scalar.dma_start(out=kt_in[c][:, sbl], in_=kst[:])
                    nc.gpsimd.collective_compute(
                        "AllGather",
                        mybir.AluOpType.bypass,
                        replica_groups=PAIRS,
                        ins=[kt_in[c][:].opt()],
                        outs=[kt_out[c][:].opt()],
                    )

                def v_chunk(c):
                    for sbl in range(2):
                        sb = c * 2 + sbl
                        vst = stage_next([128, 4, 1024])
                        for st in range(4):
                            pv = p1pv.tile([128, 1024], F32)
                            for half in range(2):
                                for dc in range(NC):
                                    nc.tensor.matmul(
                                        pv[:, half * 512 : (half + 1) * 512],
                                        lhsT=xs_all[:, sb, dc, st * 128 : (st + 1) * 128],
                                        rhs=wv[:, dc, half * 512 : (half + 1) * 512],
                                        start=(dc == 0),
                                        stop=(dc == NC - 1),
                                    )
                            nc.vector.tensor_add(vst[:, st, :], pv[:], bvb_sb[:])
                        # V stores ride the gpsimd ring, which carries nothing
                        # collective-gated ahead of them -> the V AllGathers
                        # trigger the moment the data is computed
                        nc.gpsimd.dma_start(
                            out=v_in[c][:, sbl * 4 : (sbl + 1) * 4, :], in_=vst[:]
                        )
                    nc.gpsimd.collective_compute(
                        "AllGather",
                        mybir.AluOpType.bypass,
                        replica_groups=PAIRS,
                        ins=[v_in[c][:].opt()],
                        outs=[v_out[c][:].opt()],
                    )

                # K chunks first: the exchange chain is firmware-serialized,
                # and phase 2 needs K^T chunk 0 the moment scores start while
                # V isn't consumed until ~55/~135us into attention
                kt_chunk(0)
                kt_chunk(1)
                # wq reuses wk's buffer (tag "wkq"); its WAR dependency on the
                # last K-proj matmul rides the gpsimd ring and clears before
                # the V stores behind it have data anyway
                wq = p1w.tile([128, NC, A], BF, tag="wkq")
                nc.gpsimd.dma_start(out=wq[:], in_=WqT[:])
                v_chunk(0)
                v_chunk(1)

                # prefetch the first three K^T tiles of phase 2 (sync ring;
                # gated only on the first K exchange)
                for (hh, half) in ((0, 0), (0, 1), (1, 0)):
                    ks = p2k.tile([128, NC, 512], BF, name=f"ks0_0{hh}{half}", tag="ks")
                    nc.sync.dma_start(out=ks[:], in_=kt_out[0][hh, :, half])
                    ks_pre[(0, 0, hh, half)] = ks

                # --- Q projection (overlaps the V exchanges) ---
                # qb 0..2 stage + store + (for 0,1) reload-prefetch; the LAST
                # block writes its epilogue STRAIGHT into a pre-reserved p2q
                # tile: no staging, no QT store, no reload.  This pulls the
                # last sync-queue DMA to ~10us before the V1 AllGather opens,
                # so the phase-2 pool barrier (which waits on all DMA issued
                # before the phase-1 pool exit) clears the moment the PE
                # finishes, instead of crawling through the collective.
                for qb in range(NQB):
                    if qb == NQB - 1:
                        qst = p2q.tile([128, NC, QB], BF, name=f"qt{qb}", tag="qt")
                        qt_pre[qb] = qst
                    else:
                        qst = stage_next([128, NA, 512])
                    for am in range(NA):
                        pq = p1pk.tile([128, 512], F32)
                        for dc in range(NC):
                            nc.tensor.matmul(
                                pq[:],
                                lhsT=wq[:, dc, am * 128 : (am + 1) * 128],
                                rhs=xs_all[:, qb, dc, :],
                                start=(dc == 0),
                                stop=(dc == NC - 1),
                            )
                        nc.scalar.activation(
                            qst[:, am, :], pq[:], AF.Identity,
                            bias=bqc_sb[:, am : am + 1],
                        )
                    if qb < NQB - 1:
                        nc.sync.dma_start(out=QT_d[:, qb], in_=qst[:])
                        if qb < 2:
                            qt = p2q.tile([128, NC, QB], BF, name=f"qt{qb}", tag="qt")
                            nc.sync.dma_start(out=qt[:], in_=QT_d[:, qb])
                            qt_pre[qb] = qt

            # ---------------- Phase 2: attention ----------------
            with (
                tc.tile_pool(name="p2w", bufs=1) as p2w,
                tc.tile_pool(name="p2e", bufs=2) as p2e,
                tc.tile_pool(name="p2a", bufs=1) as p2a,
                tc.tile_pool(name="p2c", bufs=1) as p2c,
                tc.tile_pool(name="p2s", bufs=1) as p2s,
                tc.tile_pool(name="p2r", bufs=1) as p2r,
                tc.tile_pool(name="p2o", bufs=2) as p2o,
                tc.tile_pool(name="pps", bufs=2, space="PSUM") as pps,
                tc.tile_pool(name="ppsum", bufs=1, space="PSUM") as ppsum,
                tc.tile_pool(name="ppt", bufs=1, space="PSUM") as ppt,
                tc.tile_pool(name="ppc", bufs=2, space="PSUM") as ppc,
                tc.tile_pool(name="ppo", bufs=2, space="PSUM") as ppo,
            ):
                # Wo lives in the space freed by the phase-1 pools; it is not
                # needed until the first output projection (~110us later).
                # Split across sync+scalar so neither ring is busy >10us when
                # the first score K^T tiles start streaming.
                wo_sb = p2w.tile([128, NC, DIM], BF)
                nc.scalar.dma_start(out=wo_sb[:, 0:4, :], in_=WoT[:, 0:4, :])
                nc.sync.dma_start(out=wo_sb[:, 4:8, :], in_=WoT[:, 4:8, :])

                # gathered V -> resident SBUF.  Emitted AFTER the phase-1
                # pools exited (so the pool-exit drains don't wait on the V
                # collectives), and parked on the gpsimd queue, which carries
                # NOTHING else in phase 2: the Tile scheduler mis-models
                # collective completion times and will happily slot a
                # collective-gated trigger ahead of critical traffic on a
                # shared queue (that stalled phase 2 by 40us in v2).  On a
                # dedicated queue the dependency order (AG triggers first,
                # loads after) is forced and head-of-line blocking is free.
                # tile_wait_until pins these AFTER the phase-1 pool-exit
                # drain in the scheduler's gpsimd ordering (the sim mis-models
                # collective timing and would otherwise slot them before the
                # drain, making the drain -- and the phase-2 pool barrier --
                # wait on the V1 collective)
                with tc.tile_wait_until(0.5):
                    for c in range(2):
                        for hh in range(2):
                            nc.gpsimd.dma_start(
                                out=v_sb[:, c * 16 + hh * 8 : c * 16 + hh * 8 + 8, :],
                                in_=v_out[c][hh, :, 0:8, :],
                            )

                ks_ring = [0]

                def do_scores(qb):
                    if qb in qt_pre:
                        qt = qt_pre.pop(qb)
                    else:
                        qt = p2q.tile([128, NC, QB], BF, name=f"qt{qb}", tag="qt")
                        nc.scalar.dma_start(out=qt[:], in_=QT_d[:, qb])
                    et = p2e.tile([128, NK, QB], BF, name=f"et{qb}", tag="et")
                    # per-partition partial softmax denominators accumulate on
                    # the vector engine as the exp tiles appear, so the PE
                    # only pays one cross-partition matmul per block
                    acc = p2a.tile([128, QB], F32, name=f"acc{qb}", tag="acc")
                    # scores^T + exp; k-tile groups of 4 share one KT load,
                    # alternating between the sync and gpsimd rings (one ring
                    # alone can't sustain the 145GB/s the PE consumes at)
                    for c in range(2):
                        for hh in range(2):
                            for half in range(2):
                                if (qb, c, hh, half) in ks_pre:
                                    ks = ks_pre.pop((qb, c, hh, half))
                                else:
                                    ks = p2k.tile([128, NC, 512], BF, name=f"ks{qb}_{c}{hh}{half}", tag="ks")
                                    eng = nc.sync if ks_ring[0] % 2 == 0 else nc.scalar
                                    ks_ring[0] += 1
                                    eng.dma_start(out=ks[:], in_=kt_out[c][hh, :, half])
                                ebase = c * 16 + hh * 8 + half * 4
                                for kt4 in range(4):
                                    ps = pps.tile([128, QB], F32, name=f"ps{qb}_{ebase+kt4}", tag="ps")
                                    for ac in range(NC):
                                        nc.tensor.matmul(
                                            ps[:],
                                            lhsT=ks[:, ac, kt4 * 128 : (kt4 + 1) * 128],
                                            rhs=qt[:, ac, :],
                                            start=(ac == 0),
                                            stop=(ac == NC - 1),
                                        )
                                    nc.scalar.activation(
                                        et[:, ebase + kt4, :],
                                        ps[:],
                                        AF.Exp,
                                        scale=float(SCALE),
                                    )
                                    kt = ebase + kt4
                                    if kt == 0:
                                        nc.vector.tensor_copy(acc[:], et[:, 0, :])
                                    else:
                                        nc.vector.tensor_add(
                                            acc[:], acc[:], et[:, kt, :]
                                        )
                    return et, acc

                def do_sums(qb, acc):
                    # softmax denominators: single cross-partition matmul on
                    # the DVE-accumulated partials, then row->partition
                    p_row = ppsum.tile([1, QB], F32, name=f"p_row{qb}", tag="p_row")
                    nc.tensor.matmul(
                        p_row[:],
                        lhsT=ones_k[:, 0:1],
                        rhs=acc[:],
                        start=True,
                        stop=True,
                    )
                    # bf16 denominators: 0.4% rounding on the softmax sums,
                    # well inside the error budget, and halves the tile
                    srow = p2s.tile([1, QB], BF, name=f"srow{qb}", tag="srow")
                    nc.scalar.copy(srow[:], p_row[:])
                    recips = p2r.tile([128, 4], F32, name=f"recips{qb}", tag="recips")
                    for qi in range(4):
                        ptt = ppt.tile([128, 1], F32, name=f"ptt{qb}_{qi}", tag="ptt")
                        nc.tensor.matmul(
                            ptt[:],
                            lhsT=srow[0:1, qi * 128 : (qi + 1) * 128],
                            rhs=ones_1[0:1, 0:1],
                            start=True,
                            stop=True,
                        )
                        nc.vector.reciprocal(recips[:, qi : qi + 1], ptt[:])
                    return recips, srow

                def do_ctxA(qb, et):
                    # first gather half of ctx^T, written straight into the
                    # (single) ct buffer; ctxB accumulates in place
                    ct = p2c.tile([128, NA, QB], BF, name=f"ct{qb}", tag="ct")
                    for at in range(NA):
                        pc = ppc.tile([128, QB], F32, name=f"pcA{qb}_{at}", tag="pc")
                        for kt in range(NK // 2):
                            nc.tensor.matmul(
                                pc[:],
                                lhsT=v_sb[:, kt, at * 128 : (at + 1) * 128],
                                rhs=et[:, kt, :],
                                start=(kt == 0),
                                stop=(kt == NK // 2 - 1),
                            )
                        nc.vector.tensor_copy(ct[:, at, :], pc[:])
                    return ct

                def do_ctxB(qb, et, ct):
                    for at in range(NA):
                        pc = ppc.tile([128, QB], F32, name=f"pcB{qb}_{at}", tag="pc")
                        for kt in range(NK // 2, NK):
                            nc.tensor.matmul(
                                pc[:],
                                lhsT=v_sb[:, kt, at * 128 : (at + 1) * 128],
                                rhs=et[:, kt, :],
                                start=(kt == NK // 2),
                                stop=(kt == NK - 1),
                            )
                        nc.vector.tensor_add(ct[:, at, :], pc[:], ct[:, at, :])
                    return ct

                def do_out(qb, ct, recips, srow):
                    # output projection + deferred softmax normalization.
                    # The bias rides the SAME psum accumulation as a rank-1
                    # matmul: (sum + denom*bo) * recip == sum*recip + bo, so
                    # the epilogue is a single per-partition multiply.
                    for qi in range(4):
                        for half in range(2):
                            po = ppo.tile([128, 512], F32, name=f"po{qb}_{qi}{half}", tag="po")
                            for ac in range(NC):
                                nc.tensor.matmul(
                                    po[:],
                                    lhsT=ct[:, ac, qi * 128 : (qi + 1) * 128],
                                    rhs=wo_sb[:, ac, half * 512 : (half + 1) * 512],
                                    start=(ac == 0),
                                    stop=False,
                                )
                            nc.tensor.matmul(
                                po[:],
                                lhsT=srow[0:1, qi * 128 : (qi + 1) * 128],
                                rhs=bob_sb[0:1, half * 512 : (half + 1) * 512],
                                start=False,
                                stop=True,
                            )
                            for h2 in range(2):
                                ob = p2o.tile([128, 256], F32, name=f"ob{qb}_{qi}{half}{h2}", tag="ob")
                                nc.vector.tensor_scalar(
                                    ob[:],
                                    po[:, h2 * 256 : (h2 + 1) * 256],
                                    recips[:, qi : qi + 1],
                                    None,
                                    op0=mybir.AluOpType.mult,
                                )
                                off = half * 512 + h2 * 256
                                eng = nc.sync if qi % 2 == 0 else nc.scalar
                                eng.dma_start(
                                    out=out[
                                        (qb * 4 + qi) * 128 : (qb * 4 + qi + 1) * 128,
                                        off : off + 256,
                                    ],
                                    in_=ob[:],
                                )

                # software pipeline: the next block's scores are emitted
                # between ctxA and ctxB of the current block, so the PE has
                # independent work while the ctx/out chain of the current
                # block is still settling.  do_sums sits after ctxA so its
                # matmul never waits on the tail of the DVE accumulation chain
                et0, acc0 = do_scores(0)
                cA0 = do_ctxA(0, et0)
                r0, sr0 = do_sums(0, acc0)
                et_next, acc_next = do_scores(1)
                ct0 = do_ctxB(0, et0, cA0)
                do_out(0, ct0, r0, sr0)
                for qb in range(1, NQB):
                    et, acc = et_next, acc_next
                    cA = do_ctxA(qb, et)
                    r, sr = do_sums(qb, acc)
                    if qb + 1 < NQB:
                        et_next, acc_next = do_scores(qb + 1)
                    ct = do_ctxB(qb, et, cA)
                    do_out(qb, ct, r, sr)
            qtp.__exit__(None, None, None)
            ksp.__exit__(None, None, None)

    _split_multiwaits(nc)
    return nc


_NC_CACHE = None


def _get_nc():
    global _NC_CACHE
    if _NC_CACHE is None:
        _NC_CACHE = _build()
    return _NC_CACHE


def kernel(x, Wq, bq, Wk, bk, Wv, bv, Wo, bo):
    global LAST_RESULT
    bf16 = ml_dtypes.bfloat16
    x = np.asarray(x, np.float32)

    def permw(w):
        # [out(=rows of W^T after .T), in] -> W^T [in, out] -> [128, in/128, out]
        wT = np.asarray(w, np.float32).T
        return np.ascontiguousarray(
            wT.reshape(NC, 128, wT.shape[1]).transpose(1, 0, 2)
        ).astype(bf16)

    WqTp = permw(Wq)
    WkTp = permw(Wk)
    WvTp = permw(Wv)
    WoTp = permw(Wo)
    bqc = np.ascontiguousarray(np.asarray(bq, np.float32).reshape(NA, 128).T)
    bkc = np.ascontiguousarray(np.asarray(bk, np.float32).reshape(NA, 128).T)
    bvb = np.ascontiguousarray(np.broadcast_to(np.asarray(bv, np.float32), (128, A))).astype(bf16)
    bob = np.ascontiguousarray(np.broadcast_to(np.asarray(bo, np.float32), (128, DIM))).astype(bf16)

    in_maps = []
    for c in range(N_CORES):
        b, h = c // 2, c % 2
        xTq = x[b, h * SQ : (h + 1) * SQ, :].T  # [DIM, SQ]
        # [dc*128+p, sb*512+s] -> [p, sb, dc, s]
        xp = np.ascontiguousarray(
            xTq.reshape(NC, 128, 4, 512).transpose(1, 2, 0, 3)
        ).astype(bf16)
        in_maps.append(
            {
                "xp": xp,
                "WqT": WqTp,
                "WkT": WkTp,
                "WvT": WvTp,
                "WoT": WoTp,
                "bqc": bqc,
                "bkc": bkc,
                "bvb": bvb,
                "bob": bob,
            }
        )

    nc = _get_nc()
    import os

    res = run_bass_kernel_spmd(
        nc,
        in_maps,
        core_ids=list(range(N_CORES)),
        trace=bool(os.environ.get("BASS_TRACE")),
    )
    LAST_RESULT = res

    out_full = np.empty((B, S, DIM), np.float32)
    for c in range(N_CORES):
        b, h = c // 2, c % 2
        out_full[b, h * SQ : (h + 1) * SQ, :] = np.asarray(
            res.results[c]["out"], dtype=np.float32
        )
    return out_full


# revision 32
# speedup vs baseline: 1.1431x; 1.0161x over previous
"""Single-head attention (B=4, S=4096, D=A=1024, fp32 I/O) on 8 TRN2 NeuronCores.

Sharding: core c handles batch b=c//2, sequence-half h=c%2 (2048 rows).
Each core projects Q, K^T and V for its own half only; core pairs exchange
K^T/V halves with chunked AllGathers (overlapped with projection compute), so
nothing is computed twice.  Attention then runs flash-style per 512-query
block against the full gathered sequence.

Q^T,K^T live as [A,S]-semantics (a on partitions), V as [S,A] (k on
partitions); scores are computed transposed ([k,q]); softmax normalization is
deferred to the output projection epilogue (exp without max subtraction is
safe here: scores are O(5)).  Matmul compute in bf16, accumulation fp32.
Softmax denominators accumulate on the vector engine (not the PE); one
cross-partition matmul per block finishes them.  k-tiles are enumerated in
gather order everywhere, which keeps scores, exp, sums and ctx consistent
without any index remapping.

v2 schedule notes (trace-driven):
- Initial loads split so the first K-proj matmul's deps (wk + x chunk 0)
  arrive on three parallel rings (~7.5us instead of ~16us).
- One 4-buffer epilogue staging rotation shared by K/V/Q projections: no
  staging WAR chain ever reaches the PE.
- wq rides the scalar ring so the V-exchange input stores on gpsimd are
  never head-of-line blocked (this previously delayed the V1 AllGather 21us).
- v_sb (gathered V) loads are emitted in PHASE 2 on the vector-engine queue:
  the phase-1 pool-exit drains no longer wait on the V collectives, so the
  first score block starts the moment Q-projection retires instead of 21us
  later.  Nothing latency-critical sits behind them on the vector queue.
- Score K^T tiles stream on sync+gpsimd alternately (one ring saturates at
  ~140GB/s, just under the 145GB/s demand), triple-buffered.
- Output stores alternate sync/scalar so the final block's 2MB doesn't
  serialize on one ring after the last matmul.
"""

import numpy as np
import ml_dtypes

import concourse.bass as bass
import concourse.tile as tile
from concourse import mybir
from concourse.bass_utils import run_bass_kernel_spmd

BF = mybir.dt.bfloat16
F32 = mybir.dt.float32
AF = mybir.ActivationFunctionType

B, S, DIM, A = 4, 4096, 1024, 1024
SQ = S // 2          # rows handled per core (query rows and local K/V rows)
NC = DIM // 128      # d chunks
NA = A // 128        # a tiles
NK = S // 128        # k tiles (global)
QB = 512             # q block width
NQB = SQ // QB
SCALE = 1.0 / np.sqrt(np.float32(A))

N_CORES = 8
PAIRS = [[0, 1], [2, 3], [4, 5], [6, 7]]

LAST_RESULT = None   # BassKernelResults of the most recent run (for test.py)


def _split_multiwaits(nc):
    """This walrus build rejects instructions carrying more than one sem wait
    (and Drains carrying any); hoist extra waits into single-wait NoOps
    preceding the instruction on the same engine."""
    for f in nc.m.functions:
        for bb in f.blocks:
            new_insts = []
            for inst in bb.instructions:
                si = inst.sync_info
                if si is not None and si.on_wait:
                    keep = 0 if isinstance(inst, mybir.InstDrain) else 1
                    if len(si.on_wait) > keep:
                        waits = list(si.on_wait)
                        hoist, rest = waits[: len(waits) - keep], waits[len(waits) - keep :]
                        for w in hoist:
                            nop = mybir.InstNoOp(
                                name=nc.get_next_instruction_name(),
                                sync_info=mybir.SyncInfo(on_wait=[w], on_update=[]),
                                bass_nofuse=True,
                                engine=inst.engine,
                            )
                            nc.register_instruction(nop)
                            new_insts.append(nop)
                        si.on_wait.clear()
                        si.on_wait.extend(rest)
                new_insts.append(inst)
            bb.instructions[:] = new_insts


def _build():
    nc = bass.Bass()

    # all pre-permuted host-side into [partition, ...contiguous...] layout
    xp = nc.declare_dram_parameter("xp", [128, 4, NC, 512], BF, isOutput=False)
    WkT = nc.declare_dram_parameter("WkT", [128, NC, A], BF, isOutput=False)
    WqT = nc.declare_dram_parameter("WqT", [128, NC, A], BF, isOutput=False)
    WvT = nc.declare_dram_parameter("WvT", [128, NC, A], BF, isOutput=False)
    WoT = nc.declare_dram_parameter("WoT", [128, NA, DIM], BF, isOutput=False)
    bqc = nc.declare_dram_parameter("bqc", [128, NA], F32, isOutput=False)
    bkc = nc.declare_dram_parameter("bkc", [128, NA], F32, isOutput=False)
    bvb = nc.declare_dram_parameter("bvb", [128, A], BF, isOutput=False)
    bob = nc.declare_dram_parameter("bob", [128, DIM], BF, isOutput=False)
    out = nc.declare_dram_parameter("out", [SQ, DIM], F32, isOutput=True)

    with tile.TileContext(nc) as tc:
        with (
            tc.tile_pool(name="dram", bufs=1, space="DRAM") as dram,
            tc.tile_pool(name="singles", bufs=1) as singles,
        ):
            # Q^T staging: [p, qb, c, q'] so stores and reloads are contiguous
            QT_d = dram.tile([128, NQB, NC, QB], BF, name="QT_d")
            # K^T exchange: [p, half, am, k'] per chunk; V: [p, j, a]
            kt_in = [
                dram.tile([128, 2, NA, 512], BF, name=f"kt_in{c}", tag=f"kti{c}")
                for c in range(2)
            ]
            kt_out = [
                dram.tile([2, 128, 2, NA, 512], BF, name=f"kt_out{c}", tag=f"kto{c}")
                for c in range(2)
            ]
            v_in = [
                dram.tile([128, 8, A], BF, name=f"v_in{c}", tag=f"vi{c}")
                for c in range(2)
            ]
            v_out = [
                dram.tile([2, 128, 8, A], BF, name=f"v_out{c}", tag=f"vo{c}")
                for c in range(2)
            ]

            warm_in = dram.tile([1, 128], BF, name="warm_in")
            warm_out = dram.tile([2, 1, 128], BF, name="warm_out")

            v_sb = singles.tile([128, NK, A], BF)        # V resident, 8.4 MB
            bqc_sb = singles.tile([128, NA], F32)
            bob_sb = singles.tile([128, DIM], BF)
            ones_k = singles.tile([128, 1], F32)         # sums matmul lhsT
            ones_1 = singles.tile([1, 1], BF)            # row->partition matmul rhs

            # phase-2 streaming pools allocated BEFORE the phase-1 pools so
            # their SBUF addresses are disjoint from phase-1 tiles -> their
            # prefetch DMAs carry no WAR dependency on phase-1 compute
            ksp = tc.tile_pool(name="p2k", bufs=3)
            p2k = ksp.__enter__()
            qtp = tc.tile_pool(name="p2q", bufs=3)
            p2q = qtp.__enter__()

            qt_pre = {}   # phase-2 Q tiles prefetched during phase 1
            ks_pre = {}   # phase-2 K^T tiles prefetched during phase 1

            # ---------------- Phase 1: projections + K/V exchange ----------
            with (
                tc.tile_pool(name="p1w", bufs=1) as p1w,
                tc.tile_pool(name="p1x", bufs=1) as p1x,
                tc.tile_pool(name="p1st", bufs=1) as p1st,
                tc.tile_pool(name="p1pk", bufs=2, space="PSUM") as p1pk,
                tc.tile_pool(name="p1pv", bufs=2, space="PSUM") as p1pv,
            ):
                wk = p1w.tile([128, NC, A], BF, tag="wkq")
                wv = p1w.tile([128, NC, A], BF, tag="wv")
                bkc_sb = p1st.tile([128, NA], F32, tag="bkc")
                bvb_sb = p1st.tile([128, A], BF, tag="bvb")
                # all of x^T stays resident through phase 1
                xs_all = p1x.tile([128, 4, NC, 512], BF)

                # one 3-deep epilogue staging rotation shared by K/V/Q: every
                # tag reuse is >=12us after the previous store retired, so no
                # WAR ever stalls an epilogue, and the last Q block bypasses
                # staging entirely (below)
                stage_n = [0]

                def stage_next(shape):
                    i = stage_n[0] % 3
                    stage_n[0] += 1
                    return p1st.tile(
                        shape, BF, tag=f"st{i}", name=f"st_u{stage_n[0]}_{i}"
                    )

                # wake the collectives firmware immediately (absorbs the
                # ~21us cross-core launch-skew barrier + ~25us cc startup
                # while the K projection is still computing)
                nc.sync.dma_start(out=warm_in[:], in_=xp[0:1, 0, 0, 0:128])
                nc.gpsimd.collective_compute(
                    "AllGather",
                    mybir.AluOpType.bypass,
                    replica_groups=PAIRS,
                    ins=[warm_in[:].opt()],
                    outs=[warm_out[:].opt()],
                )
                # initial loads: the first K-proj block needs wk (2MB) and
                # x chunk 0 (1MB); split 1MB-per-ring so it all lands ~19us
                # in (per-ring DMA sustains only ~105GB/s).  The scalar ring
                # carries NOTHING else before the kt_in exchange stores: any
                # queued junk there delays the first AllGather and with it
                # the whole firmware-serialized exchange chain.
                nc.scalar.dma_start(out=wk[:, 0:4, :], in_=WkT[:, 0:4, :])
                nc.scalar.dma_start(out=bkc_sb[:], in_=bkc[:])
                nc.sync.dma_start(out=wk[:, 4:8, :], in_=WkT[:, 4:8, :])
                nc.gpsimd.dma_start(out=xs_all[:, 0], in_=xp[:, 0])
                nc.sync.dma_start(out=xs_all[:, 1], in_=xp[:, 1])
                nc.gpsimd.dma_start(out=xs_all[:, 2], in_=xp[:, 2])
                nc.sync.dma_start(out=xs_all[:, 3], in_=xp[:, 3])
                nc.sync.dma_start(out=bvb_sb[:], in_=bvb[:])
                nc.sync.dma_start(out=bqc_sb[:], in_=bqc[:])
                nc.sync.dma_start(out=wv[:], in_=WvT[:])
                nc.sync.dma_start(out=bob_sb[:], in_=bob[:])
                nc.vector.memset(ones_k[:], 1.0)
                nc.vector.memset(ones_1[:], 1.0)

                def kt_chunk(c):
                    for sbl in range(2):
                        sb = c * 2 + sbl
                        kst = stage_next([128, NA, 512])
                        for am in range(NA):
                            pk = p1pk.tile([128, 512], F32)
                            for dc in range(NC):
                                nc.tensor.matmul(
                                    pk[:],
                                    lhsT=wk[:, dc, am * 128 : (am + 1) * 128],
                                    rhs=xs_all[:, sb, dc, :],
                                    start=(dc == 0),
                                    stop=(dc == NC - 1),
                                )
                            nc.scalar.activation(
                                kst[:, am, :], pk[:], AF.Identity,
                                bias=bkc_sb[:, am : am + 1],
                            )
                        nc.scalar.dma_start(out=kt_in[c][:, sbl], in_=kst[:])
                    nc.gpsimd.collective_compute(
                        "AllGather",
                        mybir.AluOpType.bypass,
                        replica_groups=PAIRS,
                        ins=[kt_in[c][:].opt()],
                        outs=[kt_out[c][:].opt()],
                    )

                def v_chunk(c):
                    for sbl in range(2):
                        sb = c * 2 + sbl
                        vst = stage_next([128, 4, 1024])
                        for st in range(4):
                            pv = p1pv.tile([128, 1024], F32)
                            for half in range(2):
                                for dc in range(NC):
                                    nc.tensor.matmul(
                                        pv[:, half * 512 : (half + 1) * 512],
                                        lhsT=xs_all[:, sb, dc, st * 128 : (st + 1) * 128],
                                        rhs=wv[:, dc, half * 512 : (half + 1) * 512],
                                        start=(dc == 0),
                                        stop=(dc == NC - 1),
                                    )
                            nc.vector.tensor_add(vst[:, st, :], pv[:], bvb_sb[:])
                        # V stores ride the gpsimd ring, which carries nothing
                        # collective-gated ahead of them -> the V AllGathers
                        # trigger the moment the data is computed
                        nc.gpsimd.dma_start(
                            out=v_in[c][:, sbl * 4 : (sbl + 1) * 4, :], in_=vst[:]
                        )
                    nc.gpsimd.collective_compute(
                        "AllGather",
                        mybir.AluOpType.bypass,
                        replica_groups=PAIRS,
                        ins=[v_in[c][:].opt()],
                        outs=[v_out[c][:].opt()],
                    )

                # K chunks first: the exchange chain is firmware-serialized,
                # and phase 2 needs K^T chunk 0 the moment scores start while
                # V isn't consumed until ~55/~135us into attention
                kt_chunk(0)
                kt_chunk(1)
                # wq reuses wk's buffer (tag "wkq"); its WAR dependency on the
                # last K-proj matmul rides the gpsimd ring and clears before
                # the V stores behind it have data anyway
                wq = p1w.tile([128, NC, A], BF, tag="wkq")
                nc.gpsimd.dma_start(out=wq[:], in_=WqT[:])
                v_chunk(0)
                v_chunk(1)

                # prefetch the first three K^T tiles of phase 2 on the SCALAR
                # ring: it is idle from ~90us (kt_in stores done) until the
                # first exps, so these 3MB transfer right when the first K
                # exchange lands instead of queueing behind the Q^T stores
                # and crawling under the V1 collective
                for (hh, half) in ((0, 0), (0, 1), (1, 0)):
                    ks = p2k.tile([128, NC, 512], BF, name=f"ks0_0{hh}{half}", tag="ks")
                    nc.scalar.dma_start(out=ks[:], in_=kt_out[0][hh, :, half])
                    ks_pre[(0, 0, hh, half)] = ks

                # --- Q projection (overlaps the V exchanges) ---
                # qb 0..2 stage + store + (for 0,1) reload-prefetch; the LAST
                # block writes its epilogue STRAIGHT into a pre-reserved p2q
                # tile: no staging, no QT store, no reload.  This pulls the
                # last sync-queue DMA to ~10us before the V1 AllGather opens,
                # so the phase-2 pool barrier (which waits on all DMA issued
                # before the phase-1 pool exit) clears the moment the PE
                # finishes, instead of crawling through the collective.
                for qb in range(NQB):
                    if qb == NQB - 1:
                        qst = p2q.tile([128, NC, QB], BF, name=f"qt{qb}", tag="qt")
                        qt_pre[qb] = qst
                    else:
                        qst = stage_next([128, NA, 512])
                    for am in range(NA):
                        pq = p1pk.tile([128, 512], F32)
                        for dc in range(NC):
                            nc.tensor.matmul(
                                pq[:],
                                lhsT=wq[:, dc, am * 128 : (am + 1) * 128],
                                rhs=xs_all[:, qb, dc, :],
                                start=(dc == 0),
                                stop=(dc == NC - 1),
                            )
                        nc.scalar.activation(
                            qst[:, am, :], pq[:], AF.Identity,
                            bias=bqc_sb[:, am : am + 1],
                        )
                    if qb < NQB - 1:
                        nc.sync.dma_start(out=QT_d[:, qb], in_=qst[:])
                        if qb < 2:
                            qt = p2q.tile([128, NC, QB], BF, name=f"qt{qb}", tag="qt")
                            nc.sync.dma_start(out=qt[:], in_=QT_d[:, qb])
                            qt_pre[qb] = qt

            # ---------------- Phase 2: attention ----------------
            with (
                tc.tile_pool(name="p2w", bufs=1) as p2w,
                tc.tile_pool(name="p2e", bufs=2) as p2e,
                tc.tile_pool(name="p2a", bufs=1) as p2a,
                tc.tile_pool(name="p2c", bufs=1) as p2c,
                tc.tile_pool(name="p2s", bufs=1) as p2s,
                tc.tile_pool(name="p2r", bufs=1) as p2r,
                tc.tile_pool(name="p2o", bufs=2) as p2o,
                tc.tile_pool(name="pps", bufs=2, space="PSUM") as pps,
                tc.tile_pool(name="ppsum", bufs=1, space="PSUM") as ppsum,
                tc.tile_pool(name="ppt", bufs=1, space="PSUM") as ppt,
                tc.tile_pool(name="ppc", bufs=2, space="PSUM") as ppc,
                tc.tile_pool(name="ppo", bufs=2, space="PSUM") as ppo,
            ):
                # Wo lives in the space freed by the phase-1 pools; it is not
                # needed until the first output projection (~110us later).
                # Split across sync+scalar so neither ring is busy >10us when
                # the first score K^T tiles start streaming.
                wo_sb = p2w.tile([128, NC, DIM], BF)
                nc.scalar.dma_start(out=wo_sb[:, 0:4, :], in_=WoT[:, 0:4, :])
                nc.sync.dma_start(out=wo_sb[:, 4:8, :], in_=WoT[:, 4:8, :])

                # gathered V -> resident SBUF.  Emitted AFTER the phase-1
                # pools exited (so the pool-exit drains don't wait on the V
                # collectives), and parked on the gpsimd queue, which carries
                # NOTHING else in phase 2: the Tile scheduler mis-models
                # collective completion times and will happily slot a
                # collective-gated trigger ahead of critical traffic on a
                # shared queue (that stalled phase 2 by 40us in v2).  On a
                # dedicated queue the dependency order (AG triggers first,
                # loads after) is forced and head-of-line blocking is free.
                def emit_vloads(c):
                    # gathered V -> resident SBUF, on the gpsimd queue, which
                    # carries nothing else in phase 2.  Placement is
                    # delicate: reused-space barriers capture DMA-queue
                    # high-water marks at tile-creation time in EMISSION
                    # order, so each chunk's loads are emitted as late as
                    # possible (right before their consumer, whose RAW dep
                    # needs them emitted first) -- chunk 0 before ctxA(0),
                    # chunk 1 (V1-collective-gated, landing ~35us after
                    # phase 2 starts) after ctxA(0), so the et/acc/ct
                    # first-touches never wait on the V exchanges.
                    for hh in range(2):
                        nc.gpsimd.dma_start(
                            out=v_sb[:, c * 16 + hh * 8 : c * 16 + hh * 8 + 8, :],
                            in_=v_out[c][hh, :, 0:8, :],
                        )

                ks_ring = [0]

                def do_scores(qb):
                    if qb in qt_pre:
                        qt = qt_pre.pop(qb)
                    else:
                        qt = p2q.tile([128, NC, QB], BF, name=f"qt{qb}", tag="qt")
                        nc.scalar.dma_start(out=qt[:], in_=QT_d[:, qb])
                    et = p2e.tile([128, NK, QB], BF, name=f"et{qb}", tag="et")
                    # per-partition partial softmax denominators accumulate on
                    # the vector engine as the exp tiles appear, so the PE
                    # only pays one cross-partition matmul per block
                    acc = p2a.tile([128, QB], F32, name=f"acc{qb}", tag="acc")
                    # scores^T + exp; k-tile groups of 4 share one KT load,
                    # alternating between the sync and gpsimd rings (one ring
                    # alone can't sustain the 145GB/s the PE consumes at)
                    for c in range(2):
                        for hh in range(2):
                            for half in range(2):
                                if (qb, c, hh, half) in ks_pre:
                                    ks = ks_pre.pop((qb, c, hh, half))
                                else:
                                    ks = p2k.tile([128, NC, 512], BF, name=f"ks{qb}_{c}{hh}{half}", tag="ks")
                                    eng = nc.sync if ks_ring[0] % 2 == 0 else nc.scalar
                                    ks_ring[0] += 1
                                    eng.dma_start(out=ks[:], in_=kt_out[c][hh, :, half])
                                ebase = c * 16 + hh * 8 + half * 4
                                for kt4 in range(4):
                                    ps = pps.tile([128, QB], F32, name=f"ps{qb}_{ebase+kt4}", tag="ps")
                                    for ac in range(NC):
                                        nc.tensor.matmul(
                                            ps[:],
                                            lhsT=ks[:, ac, kt4 * 128 : (kt4 + 1) * 128],
                                            rhs=qt[:, ac, :],
                                            start=(ac == 0),
                                            stop=(ac == NC - 1),
                                        )
                                    nc.scalar.activation(
                                        et[:, ebase + kt4, :],
                                        ps[:],
                                        AF.Exp,
                                        scale=float(SCALE),
                                    )
                                    kt = ebase + kt4
                                    if kt == 0:
                                        nc.vector.tensor_copy(acc[:], et[:, 0, :])
                                    else:
                                        nc.vector.tensor_add(
                                            acc[:], acc[:], et[:, kt, :]
                                        )
                    return et, acc

                def do_sums(qb, acc):
                    # softmax denominators: single cross-partition matmul on
                    # the DVE-accumulated partials, then row->partition
                    p_row = ppsum.tile([1, QB], F32, name=f"p_row{qb}", tag="p_row")
                    nc.tensor.matmul(
                        p_row[:],
                        lhsT=ones_k[:, 0:1],
                        rhs=acc[:],
                        start=True,
                        stop=True,
                    )
                    # bf16 denominators: 0.4% rounding on the softmax sums,
                    # well inside the error budget, and halves the tile
                    srow = p2s.tile([1, QB], BF, name=f"srow{qb}", tag="srow")
                    nc.scalar.copy(srow[:], p_row[:])
                    recips = p2r.tile([128, 4], F32, name=f"recips{qb}", tag="recips")
                    for qi in range(4):
                        ptt = ppt.tile([128, 1], F32, name=f"ptt{qb}_{qi}", tag="ptt")
                        nc.tensor.matmul(
                            ptt[:],
                            lhsT=srow[0:1, qi * 128 : (qi + 1) * 128],
                            rhs=ones_1[0:1, 0:1],
                            start=True,
                            stop=True,
                        )
                        nc.vector.reciprocal(recips[:, qi : qi + 1], ptt[:])
                    return recips, srow

                def do_ctxA(qb, et):
                    # first gather half of ctx^T, written straight into the
                    # (single) ct buffer; ctxB accumulates in place
                    ct = p2c.tile([128, NA, QB], BF, name=f"ct{qb}", tag="ct")
                    for at in range(NA):
                        pc = ppc.tile([128, QB], F32, name=f"pcA{qb}_{at}", tag="pc")
                        for kt in range(NK // 2):
                            nc.tensor.matmul(
                                pc[:],
                                lhsT=v_sb[:, kt, at * 128 : (at + 1) * 128],
                                rhs=et[:, kt, :],
                                start=(kt == 0),
                                stop=(kt == NK // 2 - 1),
                            )
                        nc.vector.tensor_copy(ct[:, at, :], pc[:])
                    return ct

                def do_ctxB(qb, et, ct):
                    for at in range(NA):
                        pc = ppc.tile([128, QB], F32, name=f"pcB{qb}_{at}", tag="pc")
                        for kt in range(NK // 2, NK):
                            nc.tensor.matmul(
                                pc[:],
                                lhsT=v_sb[:, kt, at * 128 : (at + 1) * 128],
                                rhs=et[:, kt, :],
                                start=(kt == NK // 2),
                                stop=(kt == NK - 1),
                            )
                        nc.vector.tensor_add(ct[:, at, :], pc[:], ct[:, at, :])
                    return ct

                def do_out(qb, ct, recips, srow):
                    # output projection + deferred softmax normalization.
                    # The bias rides the SAME psum accumulation as a rank-1
                    # matmul: (sum + denom*bo) * recip == sum*recip + bo, so
                    # the epilogue is a single per-partition multiply.
                    for qi in range(4):
                        for half in range(2):
                            po = ppo.tile([128, 512], F32, name=f"po{qb}_{qi}{half}", tag="po")
                            for ac in range(NC):
                                nc.tensor.matmul(
                                    po[:],
                                    lhsT=ct[:, ac, qi * 128 : (qi + 1) * 128],
                                    rhs=wo_sb[:, ac, half * 512 : (half + 1) * 512],
                                    start=(ac == 0),
                                    stop=False,
                                )
                            nc.tensor.matmul(
                                po[:],
                                lhsT=srow[0:1, qi * 128 : (qi + 1) * 128],
                                rhs=bob_sb[0:1, half * 512 : (half + 1) * 512],
                                start=False,
                                stop=True,
                            )
                            for h2 in range(2):
                                ob = p2o.tile([128, 256], F32, name=f"ob{qb}_{qi}{half}{h2}", tag="ob")
                                nc.vector.tensor_scalar(
                                    ob[:],
                                    po[:, h2 * 256 : (h2 + 1) * 256],
                                    recips[:, qi : qi + 1],
                                    None,
                                    op0=mybir.AluOpType.mult,
                                )
                                off = half * 512 + h2 * 256
                                eng = nc.sync if qi % 2 == 0 else nc.scalar
                                eng.dma_start(
                                    out=out[
                                        (qb * 4 + qi) * 128 : (qb * 4 + qi + 1) * 128,
                                        off : off + 256,
                                    ],
                                    in_=ob[:],
                                )

                # software pipeline: the next block's scores are emitted
                # between ctxA and ctxB of the current block, so the PE has
                # independent work while the ctx/out chain of the current
                # block is still settling.  do_sums sits after ctxA so its
                # matmul never waits on the tail of the DVE accumulation chain
                et0, acc0 = do_scores(0)
                emit_vloads(0)
                cA0 = do_ctxA(0, et0)
                emit_vloads(1)
                r0, sr0 = do_sums(0, acc0)
                et_next, acc_next = do_scores(1)
                ct0 = do_ctxB(0, et0, cA0)
                do_out(0, ct0, r0, sr0)
                for qb in range(1, NQB):
                    et, acc = et_next, acc_next
                    cA = do_ctxA(qb, et)
                    r, sr = do_sums(qb, acc)
                    if qb + 1 < NQB:
                        et_next, acc_next = do_scores(qb + 1)
                    ct = do_ctxB(qb, et, cA)
                    do_out(qb, ct, r, sr)
            qtp.__exit__(None, None, None)
            ksp.__exit__(None, None, None)

    _split_multiwaits(nc)
    return nc


_NC_CACHE = None


def _get_nc():
    global _NC_CACHE
    if _NC_CACHE is None:
        _NC_CACHE = _build()
    return _NC_CACHE


def kernel(x, Wq, bq, Wk, bk, Wv, bv, Wo, bo):
    global LAST_RESULT
    bf16 = ml_dtypes.bfloat16
    x = np.asarray(x, np.float32)

    def permw(w):
        # [out(=rows of W^T after .T), in] -> W^T [in, out] -> [128, in/128, out]
        wT = np.asarray(w, np.float32).T
        return np.ascontiguousarray(
            wT.reshape(NC, 128, wT.shape[1]).transpose(1, 0, 2)
        ).astype(bf16)

    WqTp = permw(Wq)
    WkTp = permw(Wk)
    WvTp = permw(Wv)
    WoTp = permw(Wo)
    bqc = np.ascontiguousarray(np.asarray(bq, np.float32).reshape(NA, 128).T)
    bkc = np.ascontiguousarray(np.asarray(bk, np.float32).reshape(NA, 128).T)
    bvb = np.ascontiguousarray(np.broadcast_to(np.asarray(bv, np.float32), (128, A))).astype(bf16)
    bob = np.ascontiguousarray(np.broadcast_to(np.asarray(bo, np.float32), (128, DIM))).astype(bf16)

    in_maps = []
    for c in range(N_CORES):
        b, h = c // 2, c % 2
        xTq = x[b, h * SQ : (h + 1) * SQ, :].T  # [DIM, SQ]
        # [dc*128+p, sb*512+s] -> [p, sb, dc, s]
        xp = np.ascontiguousarray(
            xTq.reshape(NC, 128, 4, 512).transpose(1, 2, 0, 3)
        ).astype(bf16)
        in_maps.append(
            {
                "xp": xp,
                "WqT": WqTp,
                "WkT": WkTp,
                "WvT": WvTp,
                "WoT": WoTp,
                "bqc": bqc,
                "bkc": bkc,
                "bvb": bvb,
                "bob": bob,
            }
        )

    nc = _get_nc()
    import os

    res = run_bass_kernel_spmd(
        nc,
        in_maps,
        core_ids=list(range(N_CORES)),
        trace=bool(os.environ.get("BASS_TRACE")),
    )
    LAST_RESULT = res

    out_full = np.empty((B, S, DIM), np.float32)
    for c in range(N_CORES):
        b, h = c // 2, c % 2
        out_full[b, h * SQ : (h + 1) * SQ, :] = np.asarray(
            res.results[c]["out"], dtype=np.float32
        )
    return out_full
